# revision 18
# baseline (speedup 1.0000x reference)
"""Cross-attention (LayerNorm -> MHA cross-attn -> out-proj -> residual) on 8 trn2 cores.

Sharding: core c -> (batch b = c//2, query-half qh = c%2). Each core computes all 16
heads for its 512 queries against the full 1024-token context of its batch. No
collectives; output shards are disjoint row blocks.

Host-side exact refactoring (linear, done in numpy):
  - gamma folded into Wq:  Wq' = gamma[:,None] * Wq ;  qb = beta @ Wq
  - post-softmax scale 1/8 folded into Wv (power of two -> exact)
  - bo folded into the residual input: x' = x + bo
  - context shipped pre-transposed (ctx^T) and pre-cast fp16; weights pre-cast fp16

Device math per core (fp16 matmul inputs, fp32 PSUM accumulation):
  hn   = (x - mu) * rsqrt(var+eps)           fp32 stats, fp16 store
  hT   = hn^T via PE transposes (fp16)
  K^T  = Wk^T ctx^T (fp16), V = ctx Wv' (fp16 -> bf16 store)
  Q^T  = Wq'^T hn^T + qb (fp16)
  lT   = K^T_h x Q^T_h per head ([j, i] layout, fp32 PSUM)
  aT   = exp(lT) -> bf16 (no max subtraction; logits bounded ~ +-70)
  V_sb holds per head-pair [V_A(64) | ones(3) | pad | V_B(64)] (162-wide blocks):
       the attn*V matmuls with a 66-wide (A) / 34-shifted 128-wide (B) stationary
       yield the softmax denominator z as a free extra PSUM row (no ones-matmul).
  zinv: z rows -> DRAM -> stride-0 partition-broadcast DMA -> vector reciprocal
  y    = x' + vals @ Wo  (fp16 matmul, fp32 residual add)
Attention is software-pipelined over head pairs (Qproj hp+1 | logits/exp hp |
attnV+normalize hp-1) and the out-proj PSUM chunks cycle over three tile-pool
tags so their first 7 accumulation steps overlap the last normalize.
"""

import numpy as np
from contextlib import ExitStack

import concourse.bass as bass
import concourse.bacc as bacc_mod
import concourse.tile as tile
from concourse import mybir

F32 = mybir.dt.float32
F16 = mybir.dt.float16
BF16 = mybir.dt.bfloat16
AF = mybir.ActivationFunctionType
ALU = mybir.AluOpType

B, NQ, NCTX, DQ, DC = 4, 1024, 1024, 1024, 768
H, DH, INNER = 16, 64, 1024
NQS = NQ // 2          # queries per core
EPS = 1e-5
N_CORES = 8


def _body(ctx, tc, nc, consts, xp, ctxt, wq, qb, wk, wv, wo, y):
    pers = ctx.enter_context(tc.tile_pool(name="pers", bufs=1))
    stat = ctx.enter_context(tc.tile_pool(name="stat", bufs=4))
    att = ctx.enter_context(tc.tile_pool(name="att", bufs=2))
    ps = ctx.enter_context(tc.tile_pool(name="ps", bufs=2, space="PSUM"))

    # ---- persistent inputs; DMA issue order == arrival priority ----
    ctxT_sb = pers.tile([128, 6, NCTX], F16, name="ctxT")
    wk_sb = pers.tile([128, 6, INNER], F16, name="wk_sb")
    for k in range(6):
        nc.sync.dma_start(out=ctxT_sb[:, k, :], in_=ctxt[k * 128:(k + 1) * 128, :])
        nc.sync.dma_start(out=wk_sb[:, k, :], in_=wk[k * 128:(k + 1) * 128, :])
    xp_sb = pers.tile([128, 4, DQ], F32, name="xp_sb")
    xp_r = xp.rearrange("(t p) d -> p t d", p=128)
    for it in range(4):
        nc.sync.dma_start(out=xp_sb[:, it, :], in_=xp_r[:, it, :])
    ident = pers.tile([128, 128], F16, name="ident")
    nc.sync.dma_start(out=ident, in_=consts[0:128, :])
    wv_sb = pers.tile([128, 6, INNER], F16, name="wv_sb")
    for k in range(6):
        nc.sync.dma_start(out=wv_sb[:, k, :], in_=wv[k * 128:(k + 1) * 128, :])
    wq_sb = pers.tile([128, 8, INNER], F16, name="wq_sb")
    for k in range(8):
        nc.sync.dma_start(out=wq_sb[:, k, :], in_=wq[k * 128:(k + 1) * 128, :])
    qb_sb = pers.tile([128, 8], F32, name="qb_sb")
    nc.sync.dma_start(out=qb_sb, in_=qb[0, :].rearrange("(m p) -> p m", p=128))
    wo_sb = pers.tile([128, 8, DQ], F16, name="wo_sb")
    for k in range(8):
        nc.sync.dma_start(out=wo_sb[:, k, :], in_=wo[k * 128:(k + 1) * 128, :])

    eps_t = pers.tile([128, 1], F32, name="eps_t")
    nc.vector.memset(eps_t, EPS)
    zero_t = pers.tile([128, 1], F32, name="zero_t")
    nc.vector.memset(zero_t, 0.0)

    # ---- persistent activations ----
    KT = pers.tile([128, 8, NCTX], F16, name="KT")         # K^T  [inner, j]
    # V layout per 162-wide head-pair block: [V_A(0:64) | ones(64:67) | pad | V_B(98:162)]
    # A stationary = cols 0:66  (even offset+width) -> vals_A rows 0:64, z_A row 64
    # B stationary = cols 34:162 -> z_B row 32 (ones at its col 32), vals_B rows 64:128
    # (PSUM partition access must be 32-aligned; 2-byte stationary APs need even
    # element offsets/widths for 4-byte alignment.)
    V_sb = pers.tile([128, 8, 1296], BF16, name="V_sb")
    QT = pers.tile([128, 8, NQS], F16, name="QT")          # Q^T  [inner, i]
    hT = pers.tile([128, 8, NQS], F16, name="hT")          # hn^T [d, i]
    valsT = pers.tile([128, 8, NQS], F16, name="valsT")    # vals^T [inner, i]
    zscr = nc.dram_tensor("zscr", [16, 512], F32)

    for jt in range(8):
        blk = V_sb[:, jt, :].rearrange("p (q r) -> p q r", r=162)
        nc.vector.memset(blk[:, :, 64:67], 1.0)
        nc.vector.memset(blk[:, :, 67:98], 0.0)

    # ---- K^T = Wk^T @ ctx^T ----
    for m in range(8):
        pk = ps.tile([128, 2, 512], F32, tag="lg", name="pk")
        for c in range(2):
            for k in range(6):
                nc.tensor.matmul(pk[:, c, :], wk_sb[:, k, m * 128:(m + 1) * 128],
                                 ctxT_sb[:, k, c * 512:(c + 1) * 512],
                                 start=(k == 0), stop=(k == 5))
        nc.scalar.activation(out=KT[:, m, :], in_=pk.rearrange("p a b -> p (a b)"),
                             func=AF.Copy)

    # ---- LayerNorm + transpose hn ----
    for it in range(4):
        st = stat.tile([128, 2, 6], F32, tag="st", name="st")
        for sb in range(2):
            nc.vector.bn_stats(out=st[:, sb, :], in_=xp_sb[:, it, sb * 512:(sb + 1) * 512])
        mv = stat.tile([128, 2], F32, tag="mv", name="mv")
        nc.vector.bn_aggr(out=mv, in_=st)
        sd = stat.tile([128, 1], F32, tag="sd", name="sd")
        nc.scalar.activation(out=sd, in_=mv[:, 1:2], func=AF.Sqrt, bias=eps_t, scale=1.0)
        rstd = stat.tile([128, 1], F32, tag="rstd", name="rstd")
        nc.vector.reciprocal(out=rstd, in_=sd)
        nmu = stat.tile([128, 1], F32, tag="nmu", name="nmu")
        nc.vector.tensor_scalar(out=nmu, in0=mv[:, 0:1], scalar1=-1.0, scalar2=None, op0=ALU.mult)
        hn = stat.tile([128, DQ], F16, tag="hn", bufs=2, name="hn")
        nc.vector.tensor_scalar(out=hn, in0=xp_sb[:, it, :], scalar1=nmu, scalar2=rstd,
                                op0=ALU.add, op1=ALU.mult)
        for g in range(2):
            ptp = ps.tile([128, 4, 128], F16, tag="lg", name="ptph")
            for q in range(4):
                dt_ = g * 4 + q
                nc.tensor.transpose(ptp[:, q, :], hn[:, dt_ * 128:(dt_ + 1) * 128], ident)
            nc.scalar.activation(
                out=hT[:, g * 4:(g + 1) * 4, it * 128:(it + 1) * 128], in_=ptp,
                func=AF.Copy)

    # ---- V = ctx @ Wv' (interleaved into [V_A | ones | V_B] pair blocks) ----
    for jt in range(8):
        pv = ps.tile([128, 2, 512], F32, tag="lg", name="pv")
        for c in range(2):
            for k in range(6):
                nc.tensor.matmul(pv[:, c, :], ctxT_sb[:, k, jt * 128:(jt + 1) * 128],
                                 wv_sb[:, k, c * 512:(c + 1) * 512],
                                 start=(k == 0), stop=(k == 5))
        for c in range(2):
            src = pv[:, c, :].rearrange("p (q d) -> p q d", d=128)
            dst = V_sb[:, jt, c * 648:(c + 1) * 648].rearrange("p (q r) -> p q r", r=162)
            nc.scalar.activation(out=dst[:, :, 0:64], in_=src[:, :, 0:64], func=AF.Copy)
            nc.scalar.activation(out=dst[:, :, 98:162], in_=src[:, :, 64:128], func=AF.Copy)

    # ---- attention: software-pipelined over head pairs ----
    # iter hp: Q-proj(hp+1) | logits+exp(hp) | attn*V(hp-1) + normalize(hp-1)
    aTA = [None] * 8
    aTB = [None] * 8

    def q_proj(hp):
        pq = ps.tile([128, 2, 512], F32, tag="lg", name="pq")
        for k in range(8):
            nc.tensor.matmul(pq[:, 0, :], wq_sb[:, k, hp * 128:(hp + 1) * 128], hT[:, k, :],
                             start=(k == 0), stop=(k == 7))
        nc.vector.tensor_scalar(out=QT[:, hp, :], in0=pq[:, 0, :], scalar1=qb_sb[:, hp:hp + 1],
                                scalar2=None, op0=ALU.add)

    def logits_exp(hp):
        aTA[hp] = att.tile([128, 8, NQS], BF16, tag="aTA", name=f"aTA{hp}")
        aTB[hp] = att.tile([128, 8, NQS], BF16, tag="aTB", name=f"aTB{hp}")
        for g in range(4):
            plA = ps.tile([128, 2, 512], F32, tag="lg", name="plA")
            plB = ps.tile([128, 2, 512], F32, tag="lg", name="plB")
            for bb in range(2):
                jt = g * 2 + bb
                nc.tensor.matmul(plA[:, bb, :], KT[0:64, hp, jt * 128:(jt + 1) * 128],
                                 QT[0:64, hp, :], start=True, stop=True)
                nc.tensor.matmul(plB[:, bb, :], KT[64:128, hp, jt * 128:(jt + 1) * 128],
                                 QT[64:128, hp, :], start=True, stop=True)
            nc.scalar.activation(out=aTA[hp][:, g * 2:(g + 1) * 2, :], in_=plA,
                                 func=AF.Exp, bias=zero_t)
            nc.scalar.activation(out=aTB[hp][:, g * 2:(g + 1) * 2, :], in_=plB,
                                 func=AF.Exp, bias=zero_t)

    def attn_v(hp):
        c, q = hp // 4, hp % 4
        off = c * 648 + q * 162
        pvA = ps.tile([128, 512], F32, tag="pva", bufs=2, name="pvA")
        pvB = ps.tile([128, 512], F32, tag="pvb", bufs=2, name="pvB")
        for jt in range(8):
            st_, sp = jt == 0, jt == 7
            nc.tensor.matmul(pvA[0:66, :], V_sb[:, jt, off:off + 66],
                             aTA[hp][:, jt, :], start=st_, stop=sp)
            nc.tensor.matmul(pvB, V_sb[:, jt, off + 34:off + 162],
                             aTB[hp][:, jt, :], start=st_, stop=sp)
        aTA[hp] = aTB[hp] = None
        # z_A sits in pvA row 64; z_B in pvB row 32 (rows 0:32, 33:64 of pvB garbage)
        zc = att.tile([128, 512], F32, tag="zc", name=f"zc{hp}")
        nc.vector.tensor_copy(out=zc[64:65, :], in_=pvA[64:65, :])
        nc.vector.tensor_copy(out=zc[32:33, :], in_=pvB[32:33, :])
        zb = att.tile([128, 512], F32, tag="zb", name=f"zb{hp}")
        nc.sync.dma_start(out=zscr[2 * hp:2 * hp + 1, :], in_=zc[64:65, :])
        nc.sync.dma_start(out=zscr[2 * hp + 1:2 * hp + 2, :], in_=zc[32:33, :])
        for bb, (lo, hi) in enumerate(((0, 64), (64, 128))):
            src = zscr[2 * hp + bb:2 * hp + bb + 1, :]
            nc.sync.dma_start(
                out=zb[lo:hi, :],
                in_=bass.AP(tensor=src.tensor, offset=src.offset, ap=[[0, 64], [1, 512]]),
            )
        zbs = att.tile([128, 512], F32, tag="zbs", name=f"zbs{hp}")
        nc.vector.reciprocal(out=zbs, in_=zb)
        nc.vector.tensor_mul(valsT[0:64, hp, :], pvA[0:64, :], zbs[0:64, :])
        nc.vector.tensor_mul(valsT[64:128, hp, :], pvB[64:128, :], zbs[64:128, :])

    q_proj(0)
    for hp in range(8):
        if hp < 7:
            q_proj(hp + 1)
        logits_exp(hp)
        if hp > 0:
            attn_v(hp - 1)
    attn_v(7)

    # ---- out projection + residual ----
    yout = ctx.enter_context(tc.tile_pool(name="yout", bufs=2))
    y_r = y.rearrange("(t p) d -> p t d", p=128)
    for c in range(2):
        for it in range(4):
            tag = ["pva", "pvb", "lg"][(c * 4 + it) % 3]
            if tag == "lg":
                po = ps.tile([128, 2, 512], F32, tag="lg", name="po")[:, 0, :]
            else:
                po = ps.tile([128, 512], F32, tag=tag, bufs=2, name="po")
            for ct in range(8):
                nc.tensor.matmul(po, valsT[:, ct, it * 128:(it + 1) * 128],
                                 wo_sb[:, ct, c * 512:(c + 1) * 512],
                                 start=(ct == 0), stop=(ct == 7))
            yt = yout.tile([128, 512], F32, tag="yt", name="yt")
            nc.vector.tensor_add(yt, po, xp_sb[:, it, c * 512:(c + 1) * 512])
            nc.sync.dma_start(out=y_r[:, it, c * 512:(c + 1) * 512], in_=yt)


def build_nc():
    nc = bacc_mod.Bacc()
    consts = nc.dram_tensor("consts", [128, 128], F16, kind="ExternalInput")
    xp = nc.dram_tensor("xp", [NQS, DQ], F32, kind="ExternalInput")
    ctxt = nc.dram_tensor("ctxt", [DC, NCTX], F16, kind="ExternalInput")
    wq = nc.dram_tensor("wq", [DQ, INNER], F16, kind="ExternalInput")
    qb = nc.dram_tensor("qb", [1, INNER], F32, kind="ExternalInput")
    wk = nc.dram_tensor("wk", [DC, INNER], F16, kind="ExternalInput")
    wv = nc.dram_tensor("wv", [DC, INNER], F16, kind="ExternalInput")
    wo = nc.dram_tensor("wo", [INNER, DQ], F16, kind="ExternalInput")
    y = nc.dram_tensor("y", [NQS, DQ], F32, kind="ExternalOutput")
    with ExitStack() as ctx:
        tc = ctx.enter_context(tile.TileContext(nc))
        _body(ctx, tc, nc, consts, xp, ctxt, wq, qb, wk, wv, wo, y)
    nc.compile()
    return nc


def make_in_maps(x, context, Wq, Wk, Wv, Wo, bo, gamma, beta):
    x = np.asarray(x, np.float32)
    context = np.asarray(context, np.float32)
    Wq = np.asarray(Wq, np.float32)
    Wk = np.asarray(Wk, np.float32)
    Wv = np.asarray(Wv, np.float32)
    Wo = np.asarray(Wo, np.float32)
    bo = np.asarray(bo, np.float32)
    gamma = np.asarray(gamma, np.float32)
    beta = np.asarray(beta, np.float32)

    wq_f = np.ascontiguousarray((gamma[:, None] * Wq).astype(np.float16))
    qb_f = np.ascontiguousarray((beta @ Wq)[None, :].astype(np.float32))
    wk_f = np.ascontiguousarray(Wk.astype(np.float16))
    wv_f = np.ascontiguousarray((Wv * np.float32(0.125)).astype(np.float16))
    wo_f = np.ascontiguousarray(Wo.astype(np.float16))
    xp_full = x + bo  # residual with bo folded in
    consts = np.eye(128, dtype=np.float16)

    in_maps = []
    for c in range(N_CORES):
        b, qh = divmod(c, 2)
        in_maps.append({
            "consts": consts,
            "xp": np.ascontiguousarray(xp_full[b, qh * NQS:(qh + 1) * NQS, :]),
            "ctxt": np.ascontiguousarray(context[b].T.astype(np.float16)),
            "wq": wq_f, "qb": qb_f, "wk": wk_f, "wv": wv_f, "wo": wo_f,
        })
    return in_maps


_NC_CACHE = []


def kernel(x, context, Wq, Wk, Wv, Wo, bo, gamma, beta):
    from concourse.bass_utils import run_bass_kernel_spmd
    if not _NC_CACHE:
        _NC_CACHE.append(build_nc())
    nc = _NC_CACHE[0]
    in_maps = make_in_maps(x, context, Wq, Wk, Wv, Wo, bo, gamma, beta)
    res = run_bass_kernel_spmd(nc, in_maps, list(range(N_CORES)))
    y = np.empty((B, NQ, DQ), np.float32)
    for c in range(N_CORES):
        b, qh = divmod(c, 2)
        y[b, qh * NQS:(qh + 1) * NQS, :] = res.results[c]["y"]
    return y
